# revision 25
# baseline (speedup 1.0000x reference)
"""KitNET anomaly-detection ensemble (25 tiny tied-weight autoencoders) on 8 Trainium2 cores.

Layout/strategy:
  - Host prep does the gather + transpose + bf16 cast: x is reorganized into an
    AE-grouped, feature-major layout xt[feat_row, chunk, sample]; the device
    needs NO PE transposes and NO fp32->bf16 casts, and DMA traffic halves.
  - Encode / decode / group-sum are BLOCK-DIAGONAL over AE chunks (matmul cost
    on the PE is moving-dim columns only), so ~6 passes instead of 12.
  - 4-stage software pipeline over 512-sample tiles: both sigmoid instructions
    of a slot have slot-old inputs, so the ACT engine (the throughput floor)
    streams back-to-back.
  - Per-AE squared-err sums via G matmuls into a DEDICATED psum bank; sqrt is
    batched after the loop (single ACT table switch, guarded against scheduler
    hoisting) and the 25-AE reduction is a mask-matmul with rmse stationary,
    which lands y sample-major for a contiguous DMA.
  - Samples are shuffled on host (b = p*128 + sb*32 + t at position
    (t, sb*128+p)) so the y DMA writes contiguous 512B partition lines.

Fast path (hb == vb == 0, which is what the harness generates): 8-AE chunks
(contraction 128), hidden 12-packed into 3 psum banks -> enc sigmoid spans
only 1536 lanes and the 8th bank is free for S. Generic-bias path: 7-AE
chunks with a ones-row folding hb/vb into the matmuls (sigma(30)=1 trick).
"""

import sys

for _p in ("/opt/trn_rl_repo", "/opt/pypackages"):
    if _p not in sys.path:
        sys.path.append(_p)

import numpy as np

B = 131072
F = 400
N_AE = 25
KF = 16          # features per AE
H = 12           # hidden per AE
EPS = 1e-6
N_CORES = 8
BC = B // N_CORES    # 16384 samples per core
NB = 512             # batch tile (matmul moving free dim)
NT = BC // NB        # 32 tiles per core

# ---------------- fast (zero-bias) layout: 8,8,8,1 AE chunks ----------------
_FK = (128, 128, 128, 16)      # xt contraction rows per chunk
_FAE = (8, 8, 8, 1)
# encode output pieces: (chunk, blob_off, local_h0, M, bank, row0); hidden
# 12-packed: flat hidden g = 96*chunk + local -> (bank, row) = (g//128, g%128)
_ENC_P = [
    (0, 0, 0, 96, 0, 0),
    (1, 96, 0, 32, 0, 96),
    (1, 128, 32, 64, 1, 0),
    (2, 192, 0, 64, 1, 64),
    (2, 256, 64, 32, 2, 0),
    (3, 288, 0, 32, 2, 32),    # 12 real cols + 20 zero-pad cols
]
# decode contraction pieces: (chunk, blob_off, local_h0, K, bank, brow0, first, last)
_DEC_P = [
    (0, 320, 0, 96, 0, 0, True, True),
    (1, 448, 0, 32, 0, 96, True, False),
    (1, 576, 32, 64, 1, 0, False, True),
    (2, 704, 0, 64, 1, 64, True, False),
    (2, 832, 64, 32, 2, 0, False, True),
    (3, 960, 0, 12, 2, 32, True, True),
]
_GM_OFF = 1088               # gm: 4 chunks x 32 cols
_MASK_OFF = 1216
_WALL_COLS = 1217

# ---------------- generic (bias) layout: 7,7,7,4 AE chunks ------------------
_AE_CH = (7, 7, 7, 4)
_K_CH = (113, 113, 113, 65)    # features + ones row
_NF_CH = (112, 112, 112, 64)
M_ENC = 113                    # 112 padded hidden + ones-generator col
BIG = 30.0                     # sigmoid(30) == 1.0

_NC_CACHE = {}


def _build_nc_fast():
    import concourse.tile as tile
    from concourse import bacc, mybir

    f32 = mybir.dt.float32
    bf16 = mybir.dt.bfloat16
    AF = mybir.ActivationFunctionType

    nc = bacc.Bacc()
    xt_d = nc.declare_dram_parameter("xt", [128, 4, BC], bf16, isOutput=False)
    wall_d = nc.declare_dram_parameter("wall", [128, _WALL_COLS], bf16, isOutput=False)
    y_d = nc.declare_dram_parameter("y", [BC], f32, isOutput=True)

    with tile.TileContext(nc) as tc:
        with (
            tc.tile_pool(name="singles", bufs=1) as singles,
            tc.tile_pool(name="xt", bufs=5) as xt_p,
            tc.tile_pool(name="ht", bufs=2) as ht_p,
            tc.tile_pool(name="rec", bufs=2) as rec_p,
            tc.tile_pool(name="encp", bufs=1, space="PSUM") as encp_p,
            tc.tile_pool(name="decp", bufs=1, space="PSUM") as decp_p,
            tc.tile_pool(name="sp", bufs=1, space="PSUM") as sp_p,
        ):
            wall = singles.tile([128, _WALL_COLS], bf16)
            nc.sync.dma_start(out=wall, in_=wall_d[:, :])
            mask = wall[0:128, _MASK_OFF : _MASK_OFF + 1]
            sall = singles.tile([128, NT, NB], f32)
            rmse = singles.tile([128, NT, NB], bf16)
            ysb = singles.tile([128, NT * 4], f32)

            xts, hts, decps, recs = {}, {}, {}, {}

            for s in range(NT + 3):
                if s < NT:
                    xt = xt_p.tile([128, 4, NB], bf16, tag="xt")
                    nc.sync.dma_start(
                        out=xt, in_=xt_d[:, :, s * NB : (s + 1) * NB]
                    )
                    xts[s] = xt

                # ---- C1: err elementwise of tile s-3 (DVE, in place)
                te = s - 3
                if te >= 0:
                    pxt = xts.pop(te)
                    prec = recs.pop(te)
                    nc.vector.tensor_sub(prec, pxt, prec)
                    nc.vector.tensor_mul(prec, prec, prec)

                # ---- B2: dec sigmoid of tile s-2
                td = s - 2
                if 0 <= td < NT:
                    pdec = decps.pop(td)
                    rec = rec_p.tile([128, 4, NB], bf16, tag="rec")
                    nc.scalar.activation(out=rec, in_=pdec, func=AF.Sigmoid)
                    recs[td] = rec

                # ---- A: encode tile s (6 piece-matmuls, packed 3 banks)
                if s < NT:
                    encp = encp_p.tile([128, 3, NB], f32, tag="encp")
                    for c, off, h0, m, bank, r0 in _ENC_P:
                        nc.tensor.matmul(
                            encp[r0 : r0 + m, bank, :],
                            lhsT=wall[0 : _FK[c], off : off + m],
                            rhs=xts[s][0 : _FK[c], c, :],
                            start=True,
                            stop=True,
                            tile_position=(0, r0),
                        )
                    ht = ht_p.tile([128, 3, NB], bf16, tag="ht")
                    # bank2 rows 64:128 are never written/used -> two reads
                    nc.scalar.activation(
                        out=ht[:, 0:2, :], in_=encp[:, 0:2, :], func=AF.Sigmoid
                    )
                    nc.scalar.activation(
                        out=ht[0:64, 2, :], in_=encp[0:64, 2, :], func=AF.Sigmoid
                    )
                    hts[s] = ht

                # ---- C2: G-matmuls of tile s-3 into the dedicated S bank
                if te >= 0:
                    spool = sp_p.tile([128, NB], f32, tag="sp")
                    for c in range(4):
                        nc.tensor.matmul(
                            spool[32 * c : 32 * (c + 1), :],
                            lhsT=wall[
                                0 : _FK[c],
                                _GM_OFF + 32 * c : _GM_OFF + 32 * (c + 1),
                            ],
                            rhs=prec[0 : _FK[c], c, :],
                            start=True,
                            stop=True,
                            tile_position=(0, 32 * c),
                        )
                    nc.vector.tensor_copy(out=sall[:, te, :], in_=spool)

                # ---- B1: decode matmuls of tile s-1 (6 pieces, 4 banks out)
                tm = s - 1
                if 0 <= tm < NT:
                    pht = hts.pop(tm)
                    decp = decp_p.tile([128, 4, NB], f32, tag="decp")
                    for c, off, h0, k, bank, br0, first, last in _DEC_P:
                        nc.tensor.matmul(
                            decp[0:128, c, :],
                            lhsT=wall[br0 : br0 + k, off : off + 128],
                            rhs=pht[br0 : br0 + k, bank, :],
                            start=first,
                            stop=last,
                            tile_position=(br0, 0),
                        )
                    decps[tm] = decp

            # ---- phase B: batched sqrt + mask-matmul y reduction
            yp = encp_p.tile([128, 3, NB], f32, tag="encp")
            eps_g = singles.tile([128, 1], f32)
            nc.vector.tensor_scalar(
                eps_g,
                sall[:, NT - 1, 0:1],
                0.0,
                EPS,
                mybir.AluOpType.mult,
                mybir.AluOpType.add,
            )
            GRP = 2
            for g in range(NT // GRP):
                nc.scalar.activation(
                    out=rmse[:, g * GRP : (g + 1) * GRP, :],
                    in_=sall[:, g * GRP : (g + 1) * GRP, :],
                    func=AF.Sqrt,
                    bias=eps_g,
                    scale=1.0 / KF,
                )
                for tt in range(g * GRP, (g + 1) * GRP):
                    for sb in range(4):
                        f = sb * 32 + tt
                        nc.tensor.matmul(
                            yp[0:128, 0, f : f + 1],
                            lhsT=rmse[0:128, tt, sb * 128 : (sb + 1) * 128],
                            rhs=mask,
                            start=True,
                            stop=True,
                        )
            nc.vector.tensor_copy(out=ysb, in_=yp[:, 0, 0 : NT * 4])
            nc.sync.dma_start(
                out=y_d[:].rearrange("(p f) -> p f", p=128), in_=ysb
            )

    nc.compile()
    return nc


def _host_mats_fast(W, hb, vb, idx):
    import ml_dtypes

    bf16 = ml_dtypes.bfloat16
    W = np.asarray(W, np.float32)
    wall = np.zeros((128, _WALL_COLS), np.float32)

    # per-chunk dense blocks, hidden 12-packed
    wenc_c = [np.zeros((_FK[c], 96), np.float32) for c in range(4)]
    wdec_c = [np.zeros((96, 128), np.float32) for c in range(4)]
    gm_c = [np.zeros((_FK[c], 32), np.float32) for c in range(4)]
    ae0 = 0
    for c, nae in enumerate(_FAE):
        for j in range(nae):
            a = ae0 + j
            for k in range(KF):
                r = j * KF + k
                wenc_c[c][r, j * H : (j + 1) * H] = W[a, k, :]
                wdec_c[c][j * H : (j + 1) * H, j * KF + k] = W[a, k, :]
                gm_c[c][r, j] = 1.0
        ae0 += nae

    # encode pieces: lhsT cols = within-chunk hidden [h0 : h0+m] (zero-padded)
    for c, off, h0, m, bank, r0 in _ENC_P:
        blk = wenc_c[c][:, h0 : min(h0 + m, 96)]
        wall[0 : _FK[c], off : off + blk.shape[1]] = blk
    # decode pieces: blob rows at brow0 so lhsT/rhs share base partitions
    for c, off, h0, k, bank, br0, first, last in _DEC_P:
        wall[br0 : br0 + k, off : off + 128] = wdec_c[c][h0 : h0 + k, :]
    for c in range(4):
        wall[0 : _FK[c], _GM_OFF + 32 * c : _GM_OFF + 32 * (c + 1)] = gm_c[c]
    for c, nae in enumerate(_FAE):
        for j in range(nae):
            wall[32 * c + j, _MASK_OFF] = 1.0

    return {"wall": np.ascontiguousarray(wall.astype(bf16))}


def _host_x(x, idx, fae):
    """Gather + transpose + shuffle + cast -> per-core xt [128, 4, BC] bf16."""
    import ml_dtypes

    bf16 = ml_dtypes.bfloat16
    perm = np.asarray(idx).reshape(-1)
    p_ = np.arange(128)
    sb_ = np.arange(4)
    t_ = np.arange(NT)
    bidx = (
        p_[None, None, :] * 128 + sb_[None, :, None] * 32 + t_[:, None, None]
    ).reshape(-1)

    outs = []
    for c in range(N_CORES):
        xc = x[c * BC : (c + 1) * BC]
        xs = xc[bidx][:, perm]                 # [BC, 400] grouped feats
        xtc = np.zeros((128, 4, BC), np.float32)
        f0 = 0
        for ch, nae in enumerate(fae):
            nf = nae * KF
            xtc[0:nf, ch, :] = xs[:, f0 : f0 + nf].T
            f0 += nf
        outs.append(xtc)
    return outs


# ---------------------------- generic bias path -----------------------------

def _build_nc_bias():
    import concourse.tile as tile
    from concourse import bacc, mybir

    f32 = mybir.dt.float32
    bf16 = mybir.dt.bfloat16
    AF = mybir.ActivationFunctionType

    nc = bacc.Bacc()
    xt_d = nc.declare_dram_parameter("xt", [128, 4, BC], bf16, isOutput=False)
    wall_d = nc.declare_dram_parameter("wall", [128, 1029], bf16, isOutput=False)
    y_d = nc.declare_dram_parameter("y", [BC], f32, isOutput=True)

    with tile.TileContext(nc) as tc:
        with (
            tc.tile_pool(name="singles", bufs=1) as singles,
            tc.tile_pool(name="xt", bufs=5) as xt_p,
            tc.tile_pool(name="ht", bufs=2) as ht_p,
            tc.tile_pool(name="rec", bufs=2) as rec_p,
            tc.tile_pool(name="encp", bufs=1, space="PSUM") as encp_p,
            tc.tile_pool(name="decp", bufs=1, space="PSUM") as decp_p,
        ):
            wall = singles.tile([128, 1029], bf16)
            nc.sync.dma_start(out=wall, in_=wall_d[:, :])

            def wenc_ap(c):
                return wall[0 : _K_CH[c], c * M_ENC : (c + 1) * M_ENC]

            def wdec_ap(c):
                return wall[0 : _K_CH[c], 452 + c * 112 : 452 + (c + 1) * 112]

            def gm_ap(c):
                return wall[0 : _NF_CH[c], 900 + c * 32 : 900 + (c + 1) * 32]

            mask = wall[0:128, 1028:1029]
            sall = singles.tile([128, NT, NB], f32)
            rmse = singles.tile([128, NT, NB], bf16)
            ysb = singles.tile([128, NT * 4], f32)

            # static psum tiles: dependencies become address-range-granular,
            # so e.g. dec-mm c0 only waits the ACT read of bank 4, not the
            # whole-tile alloc (which used to chain it behind the S copy)
            encp = encp_p.tile([128, 4, NB], f32, tag="encp")
            decp = decp_p.tile([128, 4, NB], f32, tag="decp")

            xts, hts, recs = {}, {}, {}

            for s in range(NT + 3):
                if s < NT:
                    xt = xt_p.tile([128, 4, NB], bf16, tag="xt")
                    nc.sync.dma_start(
                        out=xt, in_=xt_d[:, :, s * NB : (s + 1) * NB]
                    )
                    xts[s] = xt

                te = s - 3
                if te >= 0:
                    pxt = xts.pop(te)
                    prec = recs.pop(te)
                    nc.vector.tensor_sub(
                        prec[0:112, :, :], pxt[0:112, :, :], prec[0:112, :, :]
                    )
                    nc.vector.tensor_mul(
                        prec[0:112, :, :], prec[0:112, :, :], prec[0:112, :, :]
                    )

                # dec sigmoid of tile s-2, split so the bank-7 part (written
                # last, behind the S-copy chain) doesn't gate banks 4-6
                td = s - 2
                if 0 <= td < NT:
                    rec = rec_p.tile([128, 4, NB], bf16, tag="rec")
                    nc.scalar.activation(
                        out=rec[0:112, 0:3, :],
                        in_=decp[0:112, 0:3, :],
                        func=AF.Sigmoid,
                    )
                    nc.scalar.activation(
                        out=rec[0:112, 3, :],
                        in_=decp[0:112, 3, :],
                        func=AF.Sigmoid,
                    )
                    recs[td] = rec

                if s < NT:
                    for c in range(4):
                        k = _K_CH[c]
                        nc.tensor.matmul(
                            encp[0:M_ENC, c, :],
                            lhsT=wenc_ap(c),
                            rhs=xts[s][0:k, c, :],
                            start=True,
                            stop=True,
                        )
                    ht = ht_p.tile([128, 4, NB], bf16, tag="ht")
                    nc.scalar.activation(
                        out=ht[0:M_ENC, :, :],
                        in_=encp[0:M_ENC, :, :],
                        func=AF.Sigmoid,
                    )
                    hts[s] = ht

                # dec matmuls c0-c2 of tile s-1 (before G so they don't queue
                # behind the S-copy chain on the PE FIFO)
                tm = s - 1
                if 0 <= tm < NT:
                    pht = hts[tm]
                    for c in range(3):
                        k = _K_CH[c]
                        nc.tensor.matmul(
                            decp[0:112, c, :],
                            lhsT=wdec_ap(c),
                            rhs=pht[0:k, c, :],
                            start=True,
                            stop=True,
                        )

                # G-matmuls + S copy of tile s-3 into decp bank 7
                if te >= 0:
                    for c in range(4):
                        kg = _NF_CH[c]
                        nc.tensor.matmul(
                            decp[32 * c : 32 * (c + 1), 3, :],
                            lhsT=gm_ap(c),
                            rhs=prec[0:kg, c, :],
                            start=True,
                            stop=True,
                            tile_position=(0, 32 * c),
                        )
                    nc.vector.tensor_copy(out=sall[:, te, :], in_=decp[:, 3, :])

                # dec matmul c3 of tile s-1 (bank 7, after the S copy)
                if 0 <= tm < NT:
                    pht = hts.pop(tm)
                    nc.tensor.matmul(
                        decp[0:112, 3, :],
                        lhsT=wdec_ap(3),
                        rhs=pht[0 : _K_CH[3], 3, :],
                        start=True,
                        stop=True,
                    )

            yp = encp
            eps_g = singles.tile([128, 1], f32)
            nc.vector.tensor_scalar(
                eps_g,
                sall[:, NT - 1, 0:1],
                0.0,
                EPS,
                mybir.AluOpType.mult,
                mybir.AluOpType.add,
            )
            GRP = 2
            for g in range(NT // GRP):
                nc.scalar.activation(
                    out=rmse[:, g * GRP : (g + 1) * GRP, :],
                    in_=sall[:, g * GRP : (g + 1) * GRP, :],
                    func=AF.Sqrt,
                    bias=eps_g,
                    scale=1.0 / KF,
                )
                for tt in range(g * GRP, (g + 1) * GRP):
                    for sb in range(4):
                        f = sb * 32 + tt
                        nc.tensor.matmul(
                            yp[0:128, 0, f : f + 1],
                            lhsT=rmse[0:128, tt, sb * 128 : (sb + 1) * 128],
                            rhs=mask,
                            start=True,
                            stop=True,
                        )
            nc.vector.tensor_copy(out=ysb, in_=yp[:, 0, 0 : NT * 4])
            nc.sync.dma_start(
                out=y_d[:].rearrange("(p f) -> p f", p=128), in_=ysb
            )

    nc.compile()
    return nc


def _host_mats_bias(W, hb, vb, idx):
    import ml_dtypes

    bf16 = ml_dtypes.bfloat16
    W = np.asarray(W, np.float32)
    hb = np.asarray(hb, np.float32)
    vb = np.asarray(vb, np.float32)

    HP = 16
    wenc = np.zeros((113, 4, M_ENC), np.float32)
    wdec = np.zeros((113, 4, 112), np.float32)
    gmat = np.zeros((112, 4, 32), np.float32)
    mask = np.zeros((128, 1), np.float32)
    ae0 = 0
    for c, nae in enumerate(_AE_CH):
        ones_r = _NF_CH[c]
        for j in range(nae):
            a = ae0 + j
            for k in range(KF):
                r = j * KF + k
                wenc[r, c, j * HP : j * HP + H] = W[a, k, :]
                wdec[j * HP : j * HP + H, c, j * KF + k] = W[a, k, :]
                gmat[r, c, j] = 1.0
                wdec[ones_r, c, j * KF + k] = vb[a, k]
            wenc[ones_r, c, j * HP : j * HP + H] = hb[a, :]
            mask[32 * c + j, 0] = 1.0
        wenc[ones_r, c, ones_r] = BIG
        ae0 += nae

    wall = np.zeros((128, 1029), np.float32)
    for c in range(4):
        wall[0:113, c * M_ENC : (c + 1) * M_ENC] = wenc[:, c, :]
        wall[0:113, 452 + c * 112 : 452 + (c + 1) * 112] = wdec[:, c, :]
        wall[0:112, 900 + c * 32 : 900 + (c + 1) * 32] = gmat[:, c, :]
    wall[:, 1028] = mask[:, 0]
    return {"wall": np.ascontiguousarray(wall.astype(bf16))}


def _host_x_pack(xts_f32, fae, with_ones):
    import ml_dtypes

    bf16 = ml_dtypes.bfloat16
    outs = []
    for xtc in xts_f32:
        if with_ones:
            for ch, nae in enumerate(fae):
                xtc[nae * KF, ch, :] = 1.0
        outs.append(np.ascontiguousarray(xtc.astype(bf16)))
    return outs


def _run(x, W, hb, vb, idx, trace=False):
    from concourse.bass_utils import run_bass_kernel_spmd

    x = np.asarray(x, np.float32)
    hb = np.asarray(hb, np.float32)
    vb = np.asarray(vb, np.float32)
    # The 8-AE-chunk "fast" layout measured slower on HW (16 vs 12 matmuls
    # per tile dominates over its dedicated S bank) — use the generic path.
    fast = False

    if fast:
        if "fast" not in _NC_CACHE:
            _NC_CACHE["fast"] = _build_nc_fast()
        nc = _NC_CACHE["fast"]
        consts = _host_mats_fast(W, hb, vb, idx)
        xts = _host_x_pack(_host_x(x, idx, _FAE), _FAE, with_ones=False)
    else:
        if "bias" not in _NC_CACHE:
            _NC_CACHE["bias"] = _build_nc_bias()
        nc = _NC_CACHE["bias"]
        consts = _host_mats_bias(W, hb, vb, idx)
        xts = _host_x_pack(_host_x(x, idx, _AE_CH), _AE_CH, with_ones=True)

    in_maps = [{"xt": xts[c], **consts} for c in range(N_CORES)]
    res = run_bass_kernel_spmd(nc, in_maps, list(range(N_CORES)), trace=trace)
    y = np.concatenate([res.results[c]["y"] for c in range(N_CORES)])
    return y, res


def kernel(x, W, hb, vb, idx):
    y, _ = _run(x, W, hb, vb, idx)
    return y
